# revision 1
# baseline (speedup 1.0000x reference)
"""Trainium2 Bass kernel for the SimOTA (YOLOX-style) criterion.

Data-parallel over the batch: 8 NeuronCores x 2 images each. Each core
computes partial sums (loss_conf, loss_cls, loss_box, num_fg) for its two
images; the host sums partials, normalizes by the global num_fg and applies
the softmax-weighted combination (mirrors the torch all_reduce of
num_foregrounds).

Self-contained: hardcodes all shapes from the problem spec.
"""
import numpy as np
import ml_dtypes

IMG = 640.0
RADIUS = 5.0
W_CONF = 1.5
W_CLS = 1.0
W_REG = 6.0
EPS = 1e-8
BIG = 1e5
NEG = -3.0e5  # selection fill; any real negcost > -2.1e5

P, J, G, C = 120, 70, 16, 80  # M = P*J = 8400 exactly
M = P * J
BC = 2          # images per core
NCORES = 8

_BUILT = None


def _build():
    import concourse.bass as bass
    import concourse.bacc as bacc
    import concourse.mybir as mybir
    import concourse.bass_isa as bass_isa
    from concourse.tile import TileContext
    from concourse.masks import make_identity

    f32 = mybir.dt.float32
    bf16 = mybir.dt.bfloat16
    i32 = mybir.dt.int32
    AT = mybir.ActivationFunctionType
    OP = mybir.AluOpType
    AX = mybir.AxisListType

    nc = bacc.Bacc("TRN2", target_bir_lowering=False, debug=False)

    conf_in = nc.dram_tensor("conf_pj", [BC, P, J], f32, kind="ExternalInput")
    cls_in = nc.dram_tensor("cls_pcj", [BC, P, C * J], bf16, kind="ExternalInput")
    glab_in = nc.dram_tensor("glab_pjg", [BC, P, J * G], bf16, kind="ExternalInput")
    box_in = nc.dram_tensor("box_pl", [BC, 4, P, J], f32, kind="ExternalInput")
    anch_in = nc.dram_tensor("anch", [3, P, J], f32, kind="ExternalInput")
    gtp_in = nc.dram_tensor("gtpack", [BC, 9 * G], f32, kind="ExternalInput")
    out_t = nc.dram_tensor("out_sums", [1, 4], f32, kind="ExternalOutput")

    def A3(ap):  # per-anchor [P, J] -> broadcast over inner g
        return ap.to_broadcast([P, J, G])

    def r3(ap):  # flat [P, J*G] -> [P, J, G]
        return ap.rearrange("p (j g) -> p j g", g=G)

    with TileContext(nc) as tc:
        with tc.tile_pool(name="const", bufs=1) as cpool, \
             tc.tile_pool(name="apool", bufs=2) as apool, \
             tc.tile_pool(name="big", bufs=4) as bpool, \
             tc.tile_pool(name="tree", bufs=4) as tpool, \
             tc.tile_pool(name="pair", bufs=7) as prpool, \
             tc.tile_pool(name="keep", bufs=1) as kpool, \
             tc.tile_pool(name="sel", bufs=1) as spool, \
             tc.tile_pool(name="s6", bufs=8) as wpool, \
             tc.tile_pool(name="psum", bufs=4, space="PSUM") as ppool:

            # ---- constants
            ident = cpool.tile([128, 128], f32)
            make_identity(nc, ident)
            c_zero = cpool.tile([128, 1], f32); nc.vector.memset(c_zero, 0.0)
            c_eps = cpool.tile([128, 1], f32); nc.vector.memset(c_eps, EPS)
            c_1pe = cpool.tile([128, 1], f32); nc.vector.memset(c_1pe, 1.0 + EPS)
            c_one = cpool.tile([128, 1], f32); nc.vector.memset(c_one, 1.0)
            iota24i = cpool.tile([32, 24], i32)
            nc.gpsimd.iota(iota24i, pattern=[[1, 24]], base=0, channel_multiplier=0)
            iota24 = cpool.tile([32, 24], f32)
            nc.vector.tensor_copy(out=iota24, in_=iota24i)

            # ---- shared per-anchor inputs
            cxt = cpool.tile([P, J], f32); nc.sync.dma_start(out=cxt, in_=anch_in.ap()[0])
            cyt = cpool.tile([P, J], f32); nc.sync.dma_start(out=cyt, in_=anch_in.ap()[1])
            r5t = cpool.tile([P, J], f32); nc.sync.dma_start(out=r5t, in_=anch_in.ap()[2])

            tc.strict_bb_all_engine_barrier()

            # ---- cross-image tiles
            R_n = kpool.tile([32, 128], f32)
            R_i = kpool.tile([32, 128], f32)
            nc.vector.memset(R_n, NEG)
            nc.vector.memset(R_i, NEG)
            acc = kpool.tile([P, 4], f32)
            nc.vector.memset(acc, 0.0)
            keep_neg = [kpool.tile([P, J * G], f32, name=f"negc{b}") for b in range(BC)]
            keep_iou = [kpool.tile([P, J * G], f32, name=f"ioup{b}") for b in range(BC)]
            keep_gl = [kpool.tile([P, J * G], bf16, name=f"glk{b}") for b in range(BC)]
            keep_gg = [kpool.tile([P, 9 * G], f32, name=f"ggk{b}") for b in range(BC)]
            # S6 inputs, both images side by side: [P, BC*J]
            fg2 = kpool.tile([P, BC * J], f32)
            ioum2 = kpool.tile([P, BC * J], f32)
            xg2 = kpool.tile([P, BC * J], f32)
            bt2 = [kpool.tile([P, BC * J], f32, name=f"bt{k}") for k in range(4)]
            S02 = kpool.tile([P, BC * J], f32)
            conf2 = kpool.tile([P, BC * J], f32)
            sc2 = kpool.tile([P, BC * J], f32)
            px2a = [kpool.tile([P, BC * J], f32, name=f"pxk{k}") for k in range(4)]
            pa2 = kpool.tile([P, BC * J], f32)

            # ================= per-image phase 1 =================
            for b in range(BC):
                # --- per-anchor loads
                conf_t = conf2[:, b * J:(b + 1) * J]
                nc.sync.dma_start(out=conf_t, in_=conf_in.ap()[b])
                pxs = [px2a[k][:, b * J:(b + 1) * J] for k in range(4)]
                for k in range(4):
                    nc.sync.dma_start(out=pxs[k], in_=box_in.ap()[b, k])
                px1, py1, px2, py2 = pxs
                sc = sc2[:, b * J:(b + 1) * J]
                nc.scalar.activation(out=sc, in_=conf_t, func=AT.Sigmoid, bias=c_zero[:P])
                scb = apool.tile([P, J], bf16, tag="aw")
                nc.vector.tensor_copy(out=scb, in_=sc)
                pa = pa2[:, b * J:(b + 1) * J]
                tw = apool.tile([P, J], f32, tag="aw2")
                th = apool.tile([P, J], f32, tag="aw3")
                nc.vector.tensor_tensor(out=tw, in0=px2, in1=px1, op=OP.subtract)
                nc.vector.tensor_tensor(out=th, in0=py2, in1=py1, op=OP.subtract)
                nc.vector.tensor_tensor(out=pa, in0=tw, in1=th, op=OP.mult)

                # --- gt scalars broadcast to all partitions
                gtrow = apool.tile([1, 9 * G], f32, tag="gtr")
                nc.sync.dma_start(out=gtrow, in_=gtp_in.ap()[b:b + 1, :])
                GG = keep_gg[b]
                nc.gpsimd.partition_broadcast(GG, gtrow)

                def gb(v):  # per-gt value v -> [P, 1->J, G] broadcast view
                    return GG[:, v * G:(v + 1) * G] \
                        .rearrange("p (o g) -> p o g", o=1).to_broadcast([P, J, G])
                # order: 0 gx1, 1 gy1, 2 gx2, 3 gy2, 4 gcx, 5 gcy, 6 gw2, 7 gh2, 8 ga

                # --- S2: big cls pass (bf16)
                def bw(name):
                    return bpool.tile([P, C * J], bf16, tag="bigw", name=name)

                X = bw("X")
                nc.sync.dma_start(out=X, in_=cls_in.ap()[b])
                S_bf = bw("S_bf")
                nc.scalar.activation(out=S_bf, in_=X, func=AT.Sigmoid, bias=c_zero[:P])
                sp_bf = bw("sp_bf")  # = ln(1-sigmoid(x)) = -softplus(x)
                nc.scalar.activation(out=sp_bf, in_=S_bf, func=AT.Ln, bias=c_one[:P],
                                     scale=-1.0)
                q = bw("q")
                nc.vector.tensor_tensor(
                    out=q.rearrange("p (c j) -> p c j", j=J),
                    in0=S_bf.rearrange("p (c j) -> p c j", j=J),
                    in1=scb.rearrange("p (o j) -> p o j", o=1).to_broadcast([P, C, J]),
                    op=OP.mult)
                s2b = bw("s2b")
                nc.vector.tensor_tensor(out=s2b, in0=S_bf, in1=S_bf, op=OP.mult)
                z = bw("z")
                nc.vector.tensor_tensor(out=z, in0=sp_bf, in1=s2b, op=OP.mult)
                Pt = bw("Pt")
                nc.scalar.activation(out=Pt, in_=q, func=AT.Sqrt, bias=c_zero[:P])
                T = bw("T")
                nc.vector.tensor_scalar(out=T, in0=Pt, scalar1=-1.0, scalar2=1.0 + EPS,
                                        op0=OP.mult, op1=OP.add)

                # product tree for bneg (groups of 8), then ln + sum
                def tb(name, n):
                    return tpool.tile([P, n], bf16, tag="treeb", name=name,
                                      padded_shape=[P, 40 * J])

                T1 = tb("T1", 40 * J)
                nc.vector.tensor_tensor(out=T1, in0=T[:, :40 * J], in1=T[:, 40 * J:], op=OP.mult)
                T2 = tb("T2", 20 * J)
                nc.vector.tensor_tensor(out=T2, in0=T1[:, :20 * J], in1=T1[:, 20 * J:], op=OP.mult)
                T3 = tb("T3", 10 * J)
                nc.vector.tensor_tensor(out=T3, in0=T2[:, :10 * J], in1=T2[:, 10 * J:], op=OP.mult)

                def tf(name, n):
                    return tpool.tile([P, n], f32, tag="treef", name=name,
                                      padded_shape=[P, 10 * J])

                L = tf("L", 10 * J)
                nc.scalar.activation(out=L, in_=T3, func=AT.Ln, bias=c_zero[:P])
                A1 = tf("A1", 5 * J)
                nc.vector.tensor_tensor(out=A1, in0=L[:, :5 * J], in1=L[:, 5 * J:], op=OP.add)
                B1 = tf("B1", 2 * J)
                nc.vector.tensor_tensor(out=B1, in0=A1[:, :2 * J], in1=A1[:, 2 * J:4 * J], op=OP.add)
                B2 = tf("B2", J)
                nc.vector.tensor_tensor(out=B2, in0=B1[:, :J], in1=B1[:, J:], op=OP.add)
                nbs = apool.tile([P, J], f32, bufs=1, name=f"nbs{b}")
                nc.vector.tensor_tensor(out=nbs, in0=B2, in1=A1[:, 4 * J:], op=OP.add)

                # S0 sum tree
                Z1 = tb("Z1", 40 * J)
                nc.vector.tensor_tensor(out=Z1, in0=z[:, :40 * J], in1=z[:, 40 * J:], op=OP.add)
                Z2 = tb("Z2", 20 * J)
                nc.vector.tensor_tensor(out=Z2, in0=Z1[:, :20 * J], in1=Z1[:, 20 * J:], op=OP.add)
                Z3 = tf("Z3", 10 * J)
                nc.vector.tensor_tensor(out=Z3, in0=Z2[:, :10 * J], in1=Z2[:, 10 * J:], op=OP.add)
                Z4 = tf("Z4", 5 * J)
                nc.vector.tensor_tensor(out=Z4, in0=Z3[:, :5 * J], in1=Z3[:, 5 * J:], op=OP.add)
                Z5 = tf("Z5", 2 * J)
                nc.vector.tensor_tensor(out=Z5, in0=Z4[:, :2 * J], in1=Z4[:, 2 * J:4 * J], op=OP.add)
                Z6 = tf("Z6", J)
                nc.vector.tensor_tensor(out=Z6, in0=Z5[:, :J], in1=Z5[:, J:], op=OP.add)
                Z7 = tf("Z7", J)
                nc.vector.tensor_tensor(out=Z7, in0=Z6, in1=Z4[:, 4 * J:], op=OP.add)
                nc.vector.tensor_scalar(out=S02[:, b * J:(b + 1) * J], in0=Z7,
                                        scalar1=-0.75, scalar2=None, op0=OP.mult)

                # --- S3 + S4 pairwise tiles (shared tag "pw")
                def pw(name):
                    return prpool.tile([P, J * G], f32, tag="pw", name=name)

                # delta path on gathered label logits
                GL = keep_gl[b]
                nc.sync.dma_start(out=GL, in_=glab_in.ap()[b])
                SG = prpool.tile([P, J * G], bf16, tag="pwh", name="SG", bufs=2)
                nc.scalar.activation(out=SG, in_=GL, func=AT.Sigmoid, bias=c_zero[:P])
                qg = prpool.tile([P, J * G], bf16, tag="pwh2", name="qg", bufs=2)
                nc.vector.tensor_tensor(out=r3(qg), in0=r3(SG), in1=A3(scb), op=OP.mult)
                PG = pw("PG")
                nc.scalar.activation(out=PG, in_=qg, func=AT.Sqrt, bias=c_zero[:P])
                lp = pw("lp")
                nc.scalar.activation(out=lp, in_=PG, func=AT.Ln, bias=c_eps[:P])
                l1 = pw("l1")
                nc.scalar.activation(out=l1, in_=PG, func=AT.Ln, bias=c_1pe[:P], scale=-1.0)
                nd = prpool.tile([P, J * G], f32, bufs=1, name=f"nd{b}")
                nc.vector.tensor_tensor(out=nd, in0=lp, in1=l1, op=OP.subtract)

                # geometry masks
                dxa = pw("dxa")
                nc.vector.tensor_tensor(out=r3(dxa), in0=A3(cxt), in1=gb(4), op=OP.subtract)
                nc.scalar.activation(out=dxa, in_=dxa, func=AT.Abs, bias=c_zero[:P])
                dya = pw("dya")
                nc.vector.tensor_tensor(out=r3(dya), in0=A3(cyt), in1=gb(5), op=OP.subtract)
                nc.scalar.activation(out=dya, in_=dya, func=AT.Abs, bias=c_zero[:P])
                u1 = pw("u1")
                nc.vector.tensor_tensor(out=r3(u1), in0=r3(dxa), in1=gb(6), op=OP.is_lt)
                u2 = pw("u2")
                nc.vector.tensor_tensor(out=r3(u2), in0=r3(dya), in1=gb(7), op=OP.is_lt)
                nc.vector.tensor_tensor(out=u1, in0=u1, in1=u2, op=OP.mult)  # u1 = ib
                v1 = pw("v1")
                nc.vector.tensor_tensor(out=r3(v1), in0=r3(dxa), in1=A3(r5t), op=OP.is_lt)
                v2 = pw("v2")
                nc.vector.tensor_tensor(out=r3(v2), in0=r3(dya), in1=A3(r5t), op=OP.is_lt)
                nc.vector.tensor_tensor(out=v1, in0=v1, in1=v2, op=OP.mult)  # v1 = ic
                nc.vector.tensor_tensor(out=v2, in0=u1, in1=v1, op=OP.max)   # v2 = ib|ic
                fg0 = apool.tile([P, J], f32, bufs=1, name=f"fg0{b}")
                nc.vector.tensor_reduce(out=fg0, in_=r3(v2), axis=AX.X, op=OP.max)
                nc.vector.tensor_tensor(out=u1, in0=u1, in1=v1, op=OP.mult)  # u1 = geom

                # pairwise iou
                ex1 = pw("ex1")
                nc.vector.tensor_tensor(out=r3(ex1), in0=A3(px1), in1=gb(0), op=OP.max)
                ey1 = pw("ey1")
                nc.vector.tensor_tensor(out=r3(ey1), in0=A3(py1), in1=gb(1), op=OP.max)
                ex2 = pw("ex2")
                nc.vector.tensor_tensor(out=r3(ex2), in0=A3(px2), in1=gb(2), op=OP.min)
                ey2 = pw("ey2")
                nc.vector.tensor_tensor(out=r3(ey2), in0=A3(py2), in1=gb(3), op=OP.min)
                nc.vector.tensor_tensor(out=ex2, in0=ex2, in1=ex1, op=OP.subtract)
                nc.vector.tensor_tensor(out=ey2, in0=ey2, in1=ey1, op=OP.subtract)
                nc.scalar.activation(out=ex2, in_=ex2, func=AT.Relu, bias=c_zero[:P])
                nc.scalar.activation(out=ey2, in_=ey2, func=AT.Relu, bias=c_zero[:P])
                nc.vector.tensor_tensor(out=ex1, in0=ex2, in1=ey2, op=OP.mult)  # ex1 = inter
                nc.vector.tensor_tensor(out=r3(ey1), in0=A3(pa), in1=gb(8), op=OP.add)
                nc.vector.tensor_tensor(out=ey1, in0=ey1, in1=ex1, op=OP.subtract)  # union
                nc.vector.reciprocal(out=ey2, in_=ey1)
                iou_t = keep_iou[b]
                nc.vector.tensor_tensor(out=iou_t, in0=ex1, in1=ey2, op=OP.mult)
                ioum_p = prpool.tile([P, J * G], f32, bufs=1, name=f"ioum_p{b}")
                nc.vector.tensor_tensor(out=r3(ioum_p), in0=r3(iou_t), in1=A3(fg0), op=OP.mult)

                # negcost = 3*ln(ious_m+eps) + nd + nbs - BIG*(1-geom) - BIG*(1-fg0)
                li = pw("li")
                nc.scalar.activation(out=li, in_=ioum_p, func=AT.Ln, bias=c_eps[:P])
                nc.vector.tensor_scalar(out=li, in0=li, scalar1=3.0, scalar2=None, op0=OP.mult)
                nc.vector.tensor_tensor(out=li, in0=li, in1=nd, op=OP.add)
                nc.vector.tensor_tensor(out=r3(li), in0=r3(li), in1=A3(nbs), op=OP.add)
                nc.vector.tensor_scalar(out=u1, in0=u1, scalar1=-BIG, scalar2=BIG,
                                        op0=OP.mult, op1=OP.add)  # BIG*(1-geom)
                nc.vector.tensor_tensor(out=li, in0=li, in1=u1, op=OP.subtract)
                bfg = apool.tile([P, J], f32, tag="aw4")
                nc.vector.tensor_scalar(out=bfg, in0=fg0, scalar1=-BIG, scalar2=BIG,
                                        op0=OP.mult, op1=OP.add)
                negc = keep_neg[b]
                nc.vector.tensor_tensor(out=r3(negc), in0=r3(li), in1=A3(bfg), op=OP.subtract)

                # --- S5a: transpose + per-partition top16 + merge, per matrix
                for (src, Rdst) in ((negc, R_n), (ioum_p, R_i)):
                    TT = spool.tile([128, 1080], f32, tag="TT", bufs=1)
                    nc.vector.memset(TT[96:128, 960:1080], NEG)
                    for qk in range(9):
                        cw = 128 if qk < 8 else 96
                        pst = ppool.tile([128, P], f32, tag="pst")
                        nc.tensor.transpose(pst[:cw, :], src[:, 128 * qk:128 * qk + cw],
                                            ident[:P, :P])
                        nc.scalar.activation(out=TT[:cw, 120 * qk:120 * (qk + 1)],
                                             in_=pst[:cw, :], func=AT.Copy)
                    V = spool.tile([128, 16], f32, tag="V", bufs=2)
                    nc.vector.max(out=V[:, 0:8], in_=TT)
                    nc.vector.match_replace(out=TT, in_to_replace=V[:, 0:8],
                                            in_values=TT, imm_value=NEG)
                    nc.vector.max(out=V[:, 8:16], in_=TT)
                    for o in range(8):
                        nc.sync.dma_start(out=Rdst[16 * b:16 * (b + 1), 16 * o:16 * (o + 1)],
                                          in_=V[16 * o:16 * (o + 1), 0:16])

            # ================= merged selection =================
            T24n = spool.tile([32, 24], f32)
            T24i = spool.tile([32, 24], f32)
            for (Rsrc, Tdst) in ((R_n, T24n), (R_i, T24i)):
                nc.vector.max(out=Tdst[:, 0:8], in_=Rsrc)
                nc.vector.match_replace(out=Rsrc, in_to_replace=Tdst[:, 0:8],
                                        in_values=Rsrc, imm_value=NEG)
                nc.vector.max(out=Tdst[:, 8:16], in_=Rsrc)
                nc.vector.match_replace(out=Rsrc, in_to_replace=Tdst[:, 8:16],
                                        in_values=Rsrc, imm_value=NEG)
                nc.vector.max(out=Tdst[:, 16:24], in_=Rsrc)

            sumtop = spool.tile([32, 1], f32)
            nc.vector.tensor_reduce(out=sumtop, in_=T24i[:, 0:20], axis=AX.X, op=OP.add)
            # dyn_k-1 = max(count(iota<=sum for iota in 0..20) - 2, 0)
            cmp21 = spool.tile([32, 21], f32)
            nc.vector.tensor_scalar(out=cmp21, in0=iota24[:, 0:21], scalar1=sumtop,
                                    scalar2=None, op0=OP.is_le)
            flr = spool.tile([32, 1], f32)
            nc.vector.tensor_reduce(out=flr, in_=cmp21, axis=AX.X, op=OP.add)
            dkm1 = spool.tile([32, 1], f32)
            nc.vector.tensor_scalar(out=dkm1, in0=flr, scalar1=-2.0, scalar2=0.0,
                                    op0=OP.add, op1=OP.max)
            msk = spool.tile([32, 24], f32)
            nc.vector.tensor_scalar(out=msk, in0=iota24, scalar1=dkm1,
                                    scalar2=None, op0=OP.is_equal)
            scr = spool.tile([32, 24], f32)
            tau = spool.tile([32, 1], f32)
            nc.vector.tensor_tensor(out=scr, in0=T24n, in1=msk, op=OP.mult)
            nc.vector.tensor_reduce(out=tau, in_=scr, axis=AX.X, op=OP.add)
            tau_row = spool.tile([1, 32], f32)
            nc.sync.dma_start(out=tau_row, in_=tau)
            taug = spool.tile([P, 32], f32)
            nc.gpsimd.partition_broadcast(taug, tau_row)

            # ================= per-image phase 2: matching + reductions ====
            for b in range(BC):
                negc = keep_neg[b]
                iou_t = keep_iou[b]
                GL = keep_gl[b]
                GG = keep_gg[b]
                negc3 = r3(negc)

                def p2(name):
                    return prpool.tile([P, J * G], f32, tag="p2", name=name, bufs=3)

                m0 = p2("m0")
                nc.vector.tensor_tensor(out=r3(m0), in0=negc3,
                                        in1=taug[:, 16 * b:16 * (b + 1)]
                                        .rearrange("p (o g) -> p o g", o=1)
                                        .to_broadcast([P, J, G]),
                                        op=OP.is_ge)
                cnt = apool.tile([P, J], f32, tag="aw5")
                nc.vector.tensor_reduce(out=cnt, in_=r3(m0), axis=AX.X, op=OP.add)
                multi_e = prpool.tile([P, J * G], mybir.dt.uint32, tag="p2m",
                                      name="multi_e", bufs=2)
                nc.vector.tensor_scalar(out=r3(multi_e), in0=A3(cnt), scalar1=1.0,
                                        scalar2=None, op0=OP.is_gt)
                bm = apool.tile([P, J], f32, tag="aw7")
                nc.vector.tensor_reduce(out=bm, in_=negc3, axis=AX.X, op=OP.max)
                bmask = p2("bmask")
                nc.vector.tensor_tensor(out=r3(bmask), in0=negc3, in1=A3(bm), op=OP.is_equal)
                nc.vector.copy_predicated(m0, multi_e, bmask)  # m0 = matching

                nc.vector.tensor_reduce(out=fg2[:, b * J:(b + 1) * J], in_=r3(m0),
                                        axis=AX.X, op=OP.add)
                mm = p2("mm")
                nc.vector.tensor_tensor(out=mm, in0=m0, in1=iou_t, op=OP.mult)
                nc.vector.tensor_reduce(out=ioum2[:, b * J:(b + 1) * J], in_=r3(mm),
                                        axis=AX.X, op=OP.add)
                nc.vector.tensor_tensor(out=mm, in0=m0, in1=GL, op=OP.mult)
                nc.vector.tensor_reduce(out=xg2[:, b * J:(b + 1) * J], in_=r3(mm),
                                        axis=AX.X, op=OP.add)
                for k in range(4):
                    nc.vector.tensor_tensor(out=r3(mm), in0=r3(m0),
                                            in1=GG[:, k * G:(k + 1) * G]
                                            .rearrange("p (o g) -> p o g", o=1)
                                            .to_broadcast([P, J, G]),
                                            op=OP.mult)
                    nc.vector.tensor_reduce(out=bt2[k][:, b * J:(b + 1) * J],
                                            in_=r3(mm), axis=AX.X, op=OP.add)

            # ================= S6: elementwise losses (both images) =========
            N2 = BC * J

            def w2(name):
                return wpool.tile([P, N2], f32, tag="s6", name=name)

            fg = fg2; sc = sc2; x = conf2
            # conf focal: (0.75-0.5*fg) * (softplus(x)-x*fg) * (sc+fg-2*sc*fg)^2
            spx = w2("spx")  # sigmoid(-x)
            nc.scalar.activation(out=spx, in_=x, func=AT.Sigmoid, bias=c_zero[:P],
                                 scale=-1.0)
            nc.scalar.activation(out=spx, in_=spx, func=AT.Ln, bias=c_zero[:P])
            axf = w2("axf")
            nc.vector.tensor_tensor(out=axf, in0=x, in1=fg, op=OP.mult)
            nc.vector.tensor_tensor(out=axf, in0=spx, in1=axf, op=OP.add)  # -ce
            uu = w2("uu")
            nc.vector.tensor_tensor(out=uu, in0=sc, in1=fg, op=OP.add)
            vv = w2("vv")
            nc.vector.tensor_tensor(out=vv, in0=sc, in1=fg, op=OP.mult)
            nc.vector.tensor_scalar(out=vv, in0=vv, scalar1=-2.0, scalar2=None, op0=OP.mult)
            nc.vector.tensor_tensor(out=uu, in0=uu, in1=vv, op=OP.add)
            nc.vector.tensor_tensor(out=uu, in0=uu, in1=uu, op=OP.mult)  # (1-pt)^2
            nc.vector.tensor_scalar(out=vv, in0=fg, scalar1=0.5, scalar2=-0.75,
                                    op0=OP.mult, op1=OP.add)  # -alpha_t
            nc.vector.tensor_tensor(out=axf, in0=axf, in1=vv, op=OP.mult)
            conf_e = w2("conf_e")
            nc.vector.tensor_tensor(out=conf_e, in0=axf, in1=uu, op=OP.mult)

            # cls: fg * (S0 - 0.75*softplus(xg)*sigmoid(xg)^2 + focal(xg, ioum))
            xg = xg2; ioum = ioum2
            sxg = w2("sxg")
            nc.scalar.activation(out=sxg, in_=xg, func=AT.Sigmoid, bias=c_zero[:P])
            spg = w2("spg")  # ln(sigmoid(-xg)) = -softplus(xg)
            nc.scalar.activation(out=spg, in_=xg, func=AT.Sigmoid, bias=c_zero[:P],
                                 scale=-1.0)
            nc.scalar.activation(out=spg, in_=spg, func=AT.Ln, bias=c_zero[:P])
            l0g = w2("l0g")
            nc.vector.tensor_tensor(out=l0g, in0=sxg, in1=sxg, op=OP.mult)
            nc.vector.tensor_tensor(out=l0g, in0=spg, in1=l0g, op=OP.mult)
            nc.vector.tensor_scalar(out=l0g, in0=l0g, scalar1=-0.75, scalar2=None, op0=OP.mult)
            ceg = w2("ceg")  # -(softplus(xg) - xg*ioum) = spg' + xg*ioum
            nc.vector.tensor_tensor(out=ceg, in0=xg, in1=ioum, op=OP.mult)
            nc.vector.tensor_tensor(out=ceg, in0=spg, in1=ceg, op=OP.add)
            mq = w2("mq")
            nc.vector.tensor_tensor(out=mq, in0=sxg, in1=ioum, op=OP.mult)
            nc.vector.tensor_scalar(out=mq, in0=mq, scalar1=-2.0, scalar2=None, op0=OP.mult)
            nc.vector.tensor_tensor(out=sxg, in0=sxg, in1=ioum, op=OP.add)
            nc.vector.tensor_tensor(out=sxg, in0=sxg, in1=mq, op=OP.add)
            nc.vector.tensor_tensor(out=sxg, in0=sxg, in1=sxg, op=OP.mult)  # (1-pt)^2
            nc.vector.tensor_scalar(out=mq, in0=ioum, scalar1=0.5, scalar2=-0.75,
                                    op0=OP.mult, op1=OP.add)  # -alpha_t
            nc.vector.tensor_tensor(out=ceg, in0=ceg, in1=mq, op=OP.mult)
            nc.vector.tensor_tensor(out=ceg, in0=ceg, in1=sxg, op=OP.mult)  # focal(xg)
            nc.vector.tensor_tensor(out=l0g, in0=S02, in1=l0g, op=OP.subtract)
            nc.vector.tensor_tensor(out=l0g, in0=l0g, in1=ceg, op=OP.add)
            cls_e = w2("cls_e")
            nc.vector.tensor_tensor(out=cls_e, in0=fg, in1=l0g, op=OP.mult)

            # box: fg * (1 - giou(px, bt))
            bx1, by1, bx2, by2 = bt2
            px1a, py1a, px2b, py2b = px2a
            t1 = w2("t1"); t2_ = w2("t2_"); t3_ = w2("t3_"); t4 = w2("t4")
            nc.vector.tensor_tensor(out=t1, in0=px1a, in1=bx1, op=OP.max)
            nc.vector.tensor_tensor(out=t2_, in0=py1a, in1=by1, op=OP.max)
            nc.vector.tensor_tensor(out=t3_, in0=px2b, in1=bx2, op=OP.min)
            nc.vector.tensor_tensor(out=t4, in0=py2b, in1=by2, op=OP.min)
            nc.vector.tensor_tensor(out=t3_, in0=t3_, in1=t1, op=OP.subtract)
            nc.vector.tensor_tensor(out=t4, in0=t4, in1=t2_, op=OP.subtract)
            nc.scalar.activation(out=t3_, in_=t3_, func=AT.Relu, bias=c_zero[:P])
            nc.scalar.activation(out=t4, in_=t4, func=AT.Relu, bias=c_zero[:P])
            binter = w2("binter")
            nc.vector.tensor_tensor(out=binter, in0=t3_, in1=t4, op=OP.mult)
            nc.vector.tensor_tensor(out=t1, in0=bx2, in1=bx1, op=OP.subtract)
            nc.vector.tensor_tensor(out=t2_, in0=by2, in1=by1, op=OP.subtract)
            nc.vector.tensor_tensor(out=t1, in0=t1, in1=t2_, op=OP.mult)  # ba
            bun = w2("bun")
            nc.vector.tensor_tensor(out=bun, in0=pa2, in1=t1, op=OP.add)
            nc.vector.tensor_tensor(out=bun, in0=bun, in1=binter, op=OP.subtract)
            nc.vector.tensor_scalar(out=t1, in0=bun, scalar1=EPS, scalar2=None, op0=OP.add)
            nc.vector.reciprocal(out=t1, in_=t1)
            nc.vector.tensor_tensor(out=binter, in0=binter, in1=t1, op=OP.mult)  # biou
            nc.vector.tensor_tensor(out=t1, in0=px1a, in1=bx1, op=OP.min)
            nc.vector.tensor_tensor(out=t2_, in0=py1a, in1=by1, op=OP.min)
            nc.vector.tensor_tensor(out=t3_, in0=px2b, in1=bx2, op=OP.max)
            nc.vector.tensor_tensor(out=t4, in0=py2b, in1=by2, op=OP.max)
            nc.vector.tensor_tensor(out=t3_, in0=t3_, in1=t1, op=OP.subtract)
            nc.vector.tensor_tensor(out=t4, in0=t4, in1=t2_, op=OP.subtract)
            nc.vector.tensor_tensor(out=t3_, in0=t3_, in1=t4, op=OP.mult)  # carea
            nc.vector.tensor_tensor(out=t1, in0=t3_, in1=bun, op=OP.subtract)
            nc.vector.tensor_scalar(out=t3_, in0=t3_, scalar1=EPS, scalar2=None, op0=OP.add)
            nc.vector.reciprocal(out=t3_, in_=t3_)
            nc.vector.tensor_tensor(out=t1, in0=t1, in1=t3_, op=OP.mult)
            nc.vector.tensor_tensor(out=binter, in0=binter, in1=t1, op=OP.subtract)  # giou
            box_e = w2("box_e")
            nc.vector.tensor_scalar(out=box_e, in0=binter, scalar1=-1.0, scalar2=1.0,
                                    op0=OP.mult, op1=OP.add)
            nc.vector.tensor_tensor(out=box_e, in0=box_e, in1=fg, op=OP.mult)

            # accumulate: acc[:, k] = per-partition sums; then all-reduce + store
            for k, src in enumerate((conf_e, cls_e, box_e, fg)):
                nc.vector.tensor_reduce(out=acc[:, k:k + 1], in_=src, axis=AX.X, op=OP.add)
            accR = kpool.tile([P, 4], f32)
            nc.gpsimd.partition_all_reduce(accR, acc, channels=P,
                                           reduce_op=bass_isa.ReduceOp.add)
            nc.sync.dma_start(out=out_t.ap()[0:1, :], in_=accR[0:1, :])

    nc.finalize()
    return nc


def _get_built():
    global _BUILT
    if _BUILT is None:
        _BUILT = _build()
    return _BUILT


def _preprocess(conf_preds, cls_preds, box_preds, tgt_boxes, tgt_labels,
                anchors, strides):
    """Build per-core input maps (host-side reshapes/padding only)."""
    bf16 = ml_dtypes.bfloat16
    gt = tgt_boxes.astype(np.float32) * np.float32(IMG)
    ax = anchors[:, 0].reshape(P, J).astype(np.float32)
    ay = anchors[:, 1].reshape(P, J).astype(np.float32)
    r5 = (RADIUS * strides).reshape(P, J).astype(np.float32)
    anch = np.ascontiguousarray(np.stack([ax, ay, r5]))

    in_maps = []
    for core in range(NCORES):
        b0 = core * BC
        conf_pj = np.ascontiguousarray(
            conf_preds[b0:b0 + BC, :, 0].reshape(BC, P, J).astype(np.float32))
        cls_core = cls_preds[b0:b0 + BC]  # [BC, M, C]
        cls_pcj = np.ascontiguousarray(
            cls_core.reshape(BC, P, J, C).transpose(0, 1, 3, 2)
            .reshape(BC, P, C * J)).astype(bf16)
        glab = np.stack([
            cls_core[i][:, tgt_labels[b0 + i]] for i in range(BC)
        ])  # [BC, M, G]
        glab_pjg = np.ascontiguousarray(glab.reshape(BC, P, J * G)).astype(bf16)
        box_pl = np.ascontiguousarray(
            box_preds[b0:b0 + BC].reshape(BC, P, J, 4).transpose(0, 3, 1, 2)
            .astype(np.float32))
        gtp = np.zeros((BC, 9 * G), np.float32)
        for i in range(BC):
            g = gt[b0 + i]
            gx1, gy1, gx2, gy2 = g[:, 0], g[:, 1], g[:, 2], g[:, 3]
            vals = [gx1, gy1, gx2, gy2, (gx1 + gx2) * 0.5, (gy1 + gy2) * 0.5,
                    (gx2 - gx1) * 0.5, (gy2 - gy1) * 0.5,
                    (gx2 - gx1) * (gy2 - gy1)]
            gtp[i] = np.concatenate(vals)
        in_maps.append({
            "conf_pj": conf_pj,
            "cls_pcj": cls_pcj,
            "glab_pjg": glab_pjg,
            "box_pl": box_pl,
            "anch": anch,
            "gtpack": gtp,
        })
    return in_maps


_LAST_RESULTS = {"res": None}


def kernel(conf_preds, cls_preds, box_preds, tgt_boxes, tgt_labels,
           anchors, strides, adaptive_weight):
    from concourse.bass_utils import run_bass_kernel_spmd
    nc = _get_built()
    in_maps = _preprocess(np.asarray(conf_preds), np.asarray(cls_preds),
                          np.asarray(box_preds), np.asarray(tgt_boxes),
                          np.asarray(tgt_labels), np.asarray(anchors),
                          np.asarray(strides))
    res = run_bass_kernel_spmd(nc, in_maps, core_ids=list(range(NCORES)))
    _LAST_RESULTS["res"] = res
    sums = np.zeros(4, np.float64)
    for r in res.results:
        sums += np.asarray(r["out_sums"][0], np.float64)
    num_fg = max(sums[3], 1.0)
    lc = sums[0] / num_fg
    lcls = sums[1] / num_fg
    lbox = sums[2] / num_fg
    aw = np.asarray(adaptive_weight, np.float64)
    e = np.exp(aw - aw.max())
    w = e / e.sum()
    losses = w[0] * W_CONF * lc + w[1] * W_CLS * lcls + w[2] * W_REG * lbox
    return np.stack([lc, lcls, lbox, losses]).astype(np.float32)

